# revision 9
# baseline (speedup 1.0000x reference)
"""Segment-max normalize (DegreeOnlyFiltration) on 8 Trainium2 cores.

node_deg: (16777216,) f32, sample_pos: (8193,) int64 with uniform segment
length 2048. out[k] = node_deg[k] / max(node_deg[seg(k)]).

Sharding: data-parallel over contiguous blocks — core c owns 1024 whole
segments (2,097,152 elements). Per core the data is viewed as 8 tiles of
(128 partitions x 2048); one segment per partition row, so segment max is
a free-axis reduce and the divide is a per-partition scaled copy. No
cross-core communication.
"""

import numpy as np
from contextlib import ExitStack

import concourse.tile as tile
from concourse import bacc, mybir
from concourse.bass_utils import run_bass_kernel_spmd

N_NODES = 16_777_216
N_GRAPHS = 8192
SEG_LEN = 2048  # N_NODES // N_GRAPHS
N_CORES = 8
PER_CORE = N_NODES // N_CORES  # 2_097_152
P = 128
TILES_PER_CORE = PER_CORE // (P * SEG_LEN)  # 8 tiles of (128, 2048)

_NC_CACHE = None
LAST_RESULTS = None  # test harness hook: BassKernelResults of the last run


def _build_bass(reps=1):
    """Build the per-core Bass program.

    reps>1 repeats the full pass over the data inside one NEFF — used only
    by the timing harness to measure marginal per-pass HW time.
    """
    nc = bacc.Bacc(
        "TRN2",
        target_bir_lowering=False,
        debug=False,
        num_devices=N_CORES,
    )
    x = nc.dram_tensor(
        "x", [TILES_PER_CORE, P, SEG_LEN], mybir.dt.float32, kind="ExternalInput"
    ).ap()
    y = nc.dram_tensor(
        "y", [TILES_PER_CORE, P, SEG_LEN], mybir.dt.float32, kind="ExternalOutput"
    ).ap()
    with ExitStack() as ctx:
        tc = ctx.enter_context(tile.TileContext(nc))
        inp = ctx.enter_context(tc.tile_pool(name="inp", bufs=6))
        outp = ctx.enter_context(tc.tile_pool(name="outp", bufs=6))
        stats = ctx.enter_context(tc.tile_pool(name="stats", bufs=12))
        for _ in range(reps):
            for t in range(TILES_PER_CORE):
                tl = inp.tile([P, SEG_LEN], mybir.dt.float32)
                nc.sync.dma_start(tl[:], x[t])
                mx = stats.tile([P, 1], mybir.dt.float32)
                nc.vector.reduce_max(mx[:], tl[:], axis=mybir.AxisListType.X)
                rc = stats.tile([P, 1], mybir.dt.float32)
                nc.vector.reciprocal(rc[:], mx[:])
                ot = outp.tile([P, SEG_LEN], mybir.dt.float32)
                nc.vector.tensor_scalar_mul(ot[:], tl[:], rc[:])
                # Loads ride HWDGE (sync); stores alternate SWDGE (gpsimd)
                # and HWDGE. Splitting traffic across both descriptor paths
                # sustains ~585 GB/s/core vs ~440 single-path — reads and
                # writes contend on a shared ~550-600 GB/s port pool, and
                # the split balances the two dispatchers.
                se = nc.gpsimd if t % 2 == 0 else nc.sync
                se.dma_start(y[t], ot[:])
    nc.compile()
    return nc


def _numpy_fallback(node_deg, sample_pos):
    sp = np.asarray(sample_pos).astype(np.int64)
    n = node_deg.shape[0]
    starts = sp[:-1]
    lens = np.diff(sp)
    # segment max over non-empty segments (reduceat needs valid starts)
    valid = starts < n
    seg_max = np.full(starts.shape, -np.inf, dtype=np.float32)
    red_starts = np.minimum(starts[valid], n - 1)
    seg_max[valid] = np.maximum.reduceat(node_deg, red_starts)
    # empty segments contribute nothing; guard against len==0 garbage
    seg_max[lens <= 0] = np.inf
    per_elem = np.repeat(seg_max, np.maximum(lens, 0))[:n]
    return (node_deg / per_elem).astype(np.float32)


def kernel(node_deg, sample_pos, **_ignored):
    global _NC_CACHE, LAST_RESULTS
    node_deg = np.ascontiguousarray(node_deg, dtype=np.float32)
    sp = np.asarray(sample_pos)
    uniform = (
        node_deg.shape == (N_NODES,)
        and sp.shape == (N_GRAPHS + 1,)
        and int(sp[0]) == 0
        and int(sp[-1]) == N_NODES
        and bool(np.all(np.diff(sp) == SEG_LEN))
    )
    if not uniform:
        return _numpy_fallback(node_deg, sp)

    if _NC_CACHE is None:
        _NC_CACHE = _build_bass()
    nc = _NC_CACHE

    shards = node_deg.reshape(N_CORES, TILES_PER_CORE, P, SEG_LEN)
    in_maps = [{"x": shards[c]} for c in range(N_CORES)]
    res = run_bass_kernel_spmd(nc, in_maps, core_ids=list(range(N_CORES)))
    LAST_RESULTS = res
    out = np.concatenate([r["y"].reshape(-1) for r in res.results])
    return out.astype(np.float32, copy=False)


# revision 10
# speedup vs baseline: 1.8786x; 1.8786x over previous
"""Segment-max normalize (DegreeOnlyFiltration) on 8 Trainium2 cores.

node_deg: (16777216,) f32, sample_pos: (8193,) int64 with uniform segment
length 2048. out[k] = node_deg[k] / max(node_deg[seg(k)]).

Sharding: data-parallel over contiguous blocks — core c owns 1024 whole
segments (2,097,152 elements). Per core the data is viewed as 8 tiles of
(128 partitions x 2048); one segment per partition row, so segment max is
a free-axis reduce and the divide is a per-partition scaled copy. No
cross-core communication.
"""

import numpy as np
from contextlib import ExitStack

import concourse.tile as tile
from concourse import bacc, mybir
from concourse.bass_utils import run_bass_kernel_spmd

N_NODES = 16_777_216
N_GRAPHS = 8192
SEG_LEN = 2048  # N_NODES // N_GRAPHS
N_CORES = 8
PER_CORE = N_NODES // N_CORES  # 2_097_152
P = 128
TILES_PER_CORE = PER_CORE // (P * SEG_LEN)  # 8 tiles of (128, 2048)

_NC_CACHE = None
LAST_RESULTS = None  # test harness hook: BassKernelResults of the last run


def _build_bass(reps=1):
    """Build the per-core Bass program.

    reps>1 repeats the full pass over the data inside one NEFF — used only
    by the timing harness to measure marginal per-pass HW time.
    """
    nc = bacc.Bacc(
        "TRN2",
        target_bir_lowering=False,
        debug=False,
        num_devices=N_CORES,
    )
    x = nc.dram_tensor(
        "x", [TILES_PER_CORE, P, SEG_LEN], mybir.dt.float32, kind="ExternalInput"
    ).ap()
    y = nc.dram_tensor(
        "y", [TILES_PER_CORE, P, SEG_LEN], mybir.dt.float32, kind="ExternalOutput"
    ).ap()
    with ExitStack() as ctx:
        tc = ctx.enter_context(tile.TileContext(nc))
        inp = ctx.enter_context(tc.tile_pool(name="inp", bufs=6))
        outp = ctx.enter_context(tc.tile_pool(name="outp", bufs=6))
        stats = ctx.enter_context(tc.tile_pool(name="stats", bufs=12))
        for _ in range(reps):
            for t in range(TILES_PER_CORE):
                tl = inp.tile([P, SEG_LEN], mybir.dt.float32)
                nc.sync.dma_start(tl[:], x[t])
                mx = stats.tile([P, 1], mybir.dt.float32)
                nc.vector.reduce_max(mx[:], tl[:], axis=mybir.AxisListType.X)
                rc = stats.tile([P, 1], mybir.dt.float32)
                nc.vector.reciprocal(rc[:], mx[:])
                ot = outp.tile([P, SEG_LEN], mybir.dt.float32)
                nc.vector.tensor_scalar_mul(ot[:], tl[:], rc[:])
                # Loads ride HWDGE (sync); each store is split in half across
                # SWDGE (gpsimd) and HWDGE. Mixed read+write traffic on one
                # descriptor path caps at ~440 GB/s/core; per-store splitting
                # across both paths keeps the two dispatchers evenly fed and
                # sustains ~550 GB/s/core (measured on the memcpy floor and
                # in-kernel, same-window interleaved A/B).
                half = SEG_LEN // 2
                nc.gpsimd.dma_start(y[t][:, :half], ot[:, :half])
                nc.sync.dma_start(y[t][:, half:], ot[:, half:])
    nc.compile()
    return nc


def _numpy_fallback(node_deg, sample_pos):
    sp = np.asarray(sample_pos).astype(np.int64)
    n = node_deg.shape[0]
    starts = sp[:-1]
    lens = np.diff(sp)
    # segment max over non-empty segments (reduceat needs valid starts)
    valid = starts < n
    seg_max = np.full(starts.shape, -np.inf, dtype=np.float32)
    red_starts = np.minimum(starts[valid], n - 1)
    seg_max[valid] = np.maximum.reduceat(node_deg, red_starts)
    # empty segments contribute nothing; guard against len==0 garbage
    seg_max[lens <= 0] = np.inf
    per_elem = np.repeat(seg_max, np.maximum(lens, 0))[:n]
    return (node_deg / per_elem).astype(np.float32)


def kernel(node_deg, sample_pos, **_ignored):
    global _NC_CACHE, LAST_RESULTS
    node_deg = np.ascontiguousarray(node_deg, dtype=np.float32)
    sp = np.asarray(sample_pos)
    uniform = (
        node_deg.shape == (N_NODES,)
        and sp.shape == (N_GRAPHS + 1,)
        and int(sp[0]) == 0
        and int(sp[-1]) == N_NODES
        and bool(np.all(np.diff(sp) == SEG_LEN))
    )
    if not uniform:
        return _numpy_fallback(node_deg, sp)

    if _NC_CACHE is None:
        _NC_CACHE = _build_bass()
    nc = _NC_CACHE

    shards = node_deg.reshape(N_CORES, TILES_PER_CORE, P, SEG_LEN)
    in_maps = [{"x": shards[c]} for c in range(N_CORES)]
    res = run_bass_kernel_spmd(nc, in_maps, core_ids=list(range(N_CORES)))
    LAST_RESULTS = res
    out = np.concatenate([r["y"].reshape(-1) for r in res.results])
    return out.astype(np.float32, copy=False)
